# revision 1
# baseline (speedup 1.0000x reference)
"""Trainium2 Bass kernel for nn_ColumnEncoding (bidirectional masked LSTM
over 4096 split-delimited token segments).

Sharding: data-parallel over the 4096 columns -> 512 columns per core on 8
cores.  Embedding table and LSTM weights are replicated.  Each core runs an
identical SPMD Bass program on its shard; the host concatenates the 8
[512, 512] outputs.

Per-core device pipeline:
  1. dma_gather(transpose=True) pulls the 4096 (+special col-0) token
     embedding rows from a bf16 [VOCAB, 384] padded table straight into
     X^T layout ([emb-elem -> 3 K-tiles of 128 partitions, tokens]) in
     (step, column)-major token order.  Table column 300 is constant 1.0,
     which materializes the bias row for the fused-bias matmul.
  2. For each step t (8) and direction (fwd l=t / bwd l=7-t), gates^T
     [1024, 512cols] are accumulated in PSUM as
        W_in_aug^T @ x_l  (3 K-tiles, bias via the ones row)
      + W_hh^T     @ h_{t-1} (2 K-tiles, skipped at t=0)
     in two 4-bank PSUM units ([i,f] and [o,g] after host-side gate row
     permutation i,f,o,g).
  3. ScalarE applies sigmoid over [i|f] (one 2048-wide op) and sigmoid/tanh
     over [o]/[g]; VectorE does the fp32 cell update; h is written bf16 and
     fed back as the next matmul rhs.
  4. The ragged first column (segment length 7 instead of 8) is handled with
     per-core mask data (masked-step h/c fixups), keeping the program SPMD.
  5. Final fp32 hidden states are PE-transposed to [cols, features] and
     DMA'd out.
"""

import numpy as np
import ml_dtypes

VOCAB = 32000
EMBED = 300
HID = 256
N_COLS = 4096
SEG_LEN = 8
T = N_COLS * SEG_LEN
NCORES = 8
COLS = N_COLS // NCORES          # 512 columns per core
TOK = COLS * SEG_LEN             # 4096 gathered tokens per core
EPAD = 384                       # padded embedding row (bf16 elems, 768B)
KT_IN = 3                        # K tiles for the input matmul (384 = 3*128)
K_LAST = 45                      # valid K rows in the last input K-tile (256:300 + ones row)
KT_HH = 2                        # K tiles for the recurrent matmul (256 = 2*128)
G4 = 4 * HID                     # 1024 gates per direction

BF16 = ml_dtypes.bfloat16

_CACHE = {}


def _build_program(loop_mult=1, gather_mult=1):
    import concourse.bass as bass
    import concourse.mybir as mybir
    import concourse.tile as tile
    from concourse import bacc
    from concourse.masks import make_identity

    f32 = mybir.dt.float32
    bf16 = mybir.dt.bfloat16

    nc = bacc.Bacc("TRN2", target_bir_lowering=False, debug=False)

    emb = nc.dram_tensor("emb", [VOCAB, EPAD], bf16, kind="ExternalInput").ap()
    idx = nc.dram_tensor("idx", [128, TOK // 128], mybir.dt.int32,
                         kind="ExternalInput").ap()
    win = nc.dram_tensor("win", [2, 128, KT_IN * G4], bf16, kind="ExternalInput").ap()
    whh = nc.dram_tensor("whh", [2, 128, KT_HH * G4], bf16, kind="ExternalInput").ap()
    msk = nc.dram_tensor("msk", [2, 2 * COLS], f32, kind="ExternalInput").ap()
    out = nc.dram_tensor("out", [COLS, 2 * HID], f32, kind="ExternalOutput").ap()

    with tile.TileContext(nc) as tc:
        _body(tc, bass, mybir, make_identity, emb, idx, win, whh, msk, out,
              loop_mult, gather_mult)
    nc.compile()
    return nc


def _body(tc, bass, mybir, make_identity, emb, idx, win, whh, msk, out,
          loop_mult=1, gather_mult=1):
    nc = tc.nc
    f32 = mybir.dt.float32
    bf16 = mybir.dt.bfloat16
    SIG = mybir.ActivationFunctionType.Sigmoid
    TANH = mybir.ActivationFunctionType.Tanh
    F = 2 * COLS                 # free width of the [hid-tile, col] packed state

    with (
        tc.tile_pool(name="singles", bufs=1) as singles,
        tc.tile_pool(name="gates", bufs=2, space="PSUM") as gp,
        tc.tile_pool(name="work", bufs=2) as work,
        tc.tile_pool(name="acts", bufs=3) as acts,
    ):
        # ---- constants / inputs to SBUF ----
        idx_sb = singles.tile([128, TOK // 128], mybir.dt.int32, name="idx_sb")
        nc.sync.dma_start(out=idx_sb, in_=idx)

        win_sb = []
        whh_sb = []
        for d in range(2):
            w1 = singles.tile([128, KT_IN * G4], bf16, name=f"win_sb{d}")
            nc.sync.dma_start(out=w1, in_=win[d])
            win_sb.append(w1)
            w2 = singles.tile([128, KT_HH * G4], bf16, name=f"whh_sb{d}")
            nc.sync.dma_start(out=w2, in_=whh[d])
            whh_sb.append(w2)

        # broadcast per-core masks to all 128 partitions
        def bcast_row(r, name):
            t = singles.tile([128, F], f32, name=name)
            src = bass.AP(tensor=msk.tensor, offset=msk.offset + r * F,
                          ap=[[0, 128], [1, F]])
            nc.gpsimd.dma_start(out=t, in_=src)
            return t

        K32 = bcast_row(0, "K32")     # keep mask: 0 at core-0 col 0, else 1
        M32 = bcast_row(1, "M32")     # 1 - keep
        Kbf = singles.tile([128, F], bf16, name="Kbf")
        nc.vector.tensor_copy(Kbf, K32)

        ident = singles.tile([128, 128], f32, name="ident")
        make_identity(nc, ident)

        # ---- gather X^T per step: XT[l][p, kt, n] = emb_row(tok[l,n])[kt*128+p]
        # indirect row gathers (128 rows/op) -> per-l DRAM staging -> DMA
        # transposes back into [emb-elem, token] K-tile layout.
        XT = [None] * SEG_LEN
        with tc.tile_pool(name="gx", bufs=4) as gxp, \
             tc.tile_pool(name="xd", bufs=1, space="DRAM") as xdp:
            for g_rep in range(gather_mult):
                for l in (0, 7, 1, 6, 2, 5, 3, 4):
                    xd = xdp.tile([COLS, EPAD], bf16, name=f"xd{g_rep}_{l}",
                                  tag=f"xd{l}")
                    for jj in range(COLS // 128):
                        j = l * (COLS // 128) + jj
                        xg = gxp.tile([128, EPAD], bf16, name=f"xg{l}_{jj}",
                                      tag="xg")
                        nc.gpsimd.indirect_dma_start(
                            out=xg,
                            out_offset=None,
                            in_=emb[:, :],
                            in_offset=bass.IndirectOffsetOnAxis(
                                ap=idx_sb[:, j:j + 1], axis=0),
                        )
                        nc.sync.dma_start(out=xd[jj * 128:(jj + 1) * 128, :],
                                          in_=xg)
                    xt = singles.tile([128, KT_IN, COLS], bf16,
                                      name=f"xt{g_rep}_{l}", tag=f"xt{l}")
                    for kt in range(KT_IN):
                        nc.sync.dma_start_transpose(
                            out=xt[:, kt, :], in_=xd[:, kt * 128:(kt + 1) * 128])
                    XT[l] = xt

        # ---- recurrence ----
        h_prev = [None, None]        # bf16 [128, F] per direction
        c_prev = [None, None]        # f32  [128, F] per direction
        h_fin32 = [None, None]       # final fp32 hidden per direction
        h6_32 = None                 # fwd h after step 6 (col-0 ragged fix)

        for rep_t in range(loop_mult * SEG_LEN):
            t = rep_t % SEG_LEN
            for d in range(2):       # 0 = fwd, 1 = bwd
                l = t if d == 0 else SEG_LEN - 1 - t
                units = []
                for ui in range(2):  # unit 0: gates [i|f], unit 1: [o|g]
                    u = gp.tile([128, 4 * COLS], f32, name=f"u{t}_{d}_{ui}",
                                tag="u")
                    for mi in range(4):
                        m = ui * 4 + mi
                        dst = u[:, mi * COLS:(mi + 1) * COLS]
                        for kt in range(KT_IN):
                            kp = K_LAST if kt == KT_IN - 1 else 128
                            nc.tensor.matmul(
                                dst,
                                win_sb[d][0:kp, kt * G4 + m * 128:kt * G4 + (m + 1) * 128],
                                XT[l][0:kp, kt, :],
                                start=(kt == 0),
                                stop=(kt == KT_IN - 1 and t == 0),
                            )
                        if t > 0:
                            for kt in range(KT_HH):
                                nc.tensor.matmul(
                                    dst,
                                    whh_sb[d][:, kt * G4 + m * 128:kt * G4 + (m + 1) * 128],
                                    h_prev[d][:, kt * COLS:(kt + 1) * COLS],
                                    start=False,
                                    stop=(kt == KT_HH - 1),
                                )
                    units.append(u)

                s1 = acts.tile([128, 4 * COLS], f32, name=f"s1_{t}_{d}", tag="s1")
                nc.scalar.activation(s1, units[0][:, :], SIG)
                so = acts.tile([128, F], f32, name=f"so_{t}_{d}", tag="so")
                nc.scalar.activation(so, units[1][:, 0:F], SIG)
                tg = acts.tile([128, F], f32, name=f"tg_{t}_{d}", tag="tg")
                nc.scalar.activation(tg, units[1][:, F:2 * F], TANH)

                # cell update (fp32): c = sig_f * c + sig_i * tanh_g
                t2 = work.tile([128, F], f32, name=f"t2_{t}_{d}", tag="t2")
                nc.vector.tensor_mul(t2, s1[:, 0:F], tg)
                if t == 0:
                    c_new = t2
                else:
                    t1 = work.tile([128, F], f32, name=f"t1_{t}_{d}", tag="t1")
                    nc.vector.tensor_mul(t1, s1[:, F:2 * F], c_prev[d])
                    c_new = work.tile([128, F], f32, name=f"c_{t}_{d}", tag=f"c{d}")
                    nc.vector.tensor_add(c_new, t1, t2)

                tc_ = acts.tile([128, F], f32, name=f"tc_{t}_{d}", tag="tc")
                nc.scalar.activation(tc_, c_new, TANH)

                h_bf = work.tile([128, F], bf16, name=f"h_{t}_{d}", tag=f"h{d}")
                nc.vector.tensor_mul(h_bf, so, tc_)

                if d == 1 and t == 0:
                    # bwd step 0 is masked for (core 0) column 0: zero h, c
                    cm = work.tile([128, F], f32, name="c_bm", tag=f"c{d}")
                    nc.vector.tensor_mul(cm, c_new, K32)
                    c_new = cm
                    hm = work.tile([128, F], bf16, name="h_bm", tag=f"h{d}")
                    nc.vector.tensor_mul(hm, h_bf, Kbf)
                    h_bf = hm

                if d == 0 and t == SEG_LEN - 2:
                    # fwd h after step 6, fp32 (output for the ragged column 0)
                    h6_32 = work.tile([128, F], f32, name="h6_32", tag="hf32",
                                      bufs=6)
                    nc.vector.tensor_mul(h6_32, so, tc_)
                if t == SEG_LEN - 1:
                    hf = work.tile([128, F], f32, name=f"hfin{d}", tag="hf32",
                                   bufs=6)
                    nc.vector.tensor_mul(hf, so, tc_)
                    h_fin32[d] = hf

                c_prev[d] = c_new
                h_prev[d] = h_bf

        # fwd ragged fix: column 0 of core 0 takes the step-6 hidden state
        # (blend: h7*K + h6*(1-K); avoids copy_predicated's int-mask needs)
        b1 = work.tile([128, F], f32, name="b1", tag="hf32", bufs=6)
        nc.vector.tensor_mul(b1, h_fin32[0], K32)
        b2 = work.tile([128, F], f32, name="b2", tag="hf32", bufs=6)
        nc.vector.tensor_mul(b2, h6_32, M32)
        hf_sel = work.tile([128, F], f32, name="hf_sel", tag="hf32", bufs=6)
        nc.vector.tensor_add(hf_sel, b1, b2)
        h_fin32[0] = hf_sel

        # ---- transpose [hid, col] -> [col, feat] and write out ----
        out_t = []
        for nt in range(COLS // 128):
            o = singles.tile([128, 2 * HID], f32, name=f"out_t{nt}")
            out_t.append(o)
        for d in range(2):
            for ht in range(2):
                for nt in range(COLS // 128):
                    tp = gp.tile([128, 128], f32, name=f"tp{d}_{ht}_{nt}", tag="u")
                    nc.tensor.transpose(
                        tp, h_fin32[d][:, ht * COLS + nt * 128:ht * COLS + (nt + 1) * 128],
                        ident)
                    nc.vector.tensor_copy(
                        out_t[nt][:, d * HID + ht * 128:d * HID + (ht + 1) * 128], tp)
        for nt in range(COLS // 128):
            nc.sync.dma_start(out=out[nt * 128:(nt + 1) * 128, :], in_=out_t[nt])


def _prep_host(inputs):
    """Build the per-core input maps from the full problem inputs."""
    emb_table = np.asarray(inputs["emb_table"], dtype=np.float32)
    seq = np.asarray(inputs["seq_s"]).astype(np.int64)

    embp = np.zeros((VOCAB, EPAD), dtype=BF16)
    embp[:, :EMBED] = emb_table.astype(BF16)
    embp[:, EMBED] = 1.0  # ones column -> bias row of X^T

    perm = np.concatenate([np.arange(0, 2 * HID),            # i, f
                           np.arange(3 * HID, 4 * HID),      # o
                           np.arange(2 * HID, 3 * HID)])     # g

    def prep_win(w_ih, b_ih, b_hh):
        aug = np.zeros((G4, KT_IN * 128), dtype=np.float32)
        aug[:, :EMBED] = np.asarray(w_ih, np.float32)
        aug[:, EMBED] = np.asarray(b_ih, np.float32) + np.asarray(b_hh, np.float32)
        aug = aug[perm]
        a = aug.T.reshape(KT_IN, 128, G4).transpose(1, 0, 2)
        return np.ascontiguousarray(a.reshape(128, KT_IN * G4)).astype(BF16)

    def prep_whh(w_hh):
        a = np.asarray(w_hh, np.float32)[perm].T.reshape(KT_HH, 128, G4)
        return np.ascontiguousarray(
            a.transpose(1, 0, 2).reshape(128, KT_HH * G4)).astype(BF16)

    win_arr = np.stack([prep_win(inputs["w_ih_f"], inputs["b_ih_f"], inputs["b_hh_f"]),
                        prep_win(inputs["w_ih_b"], inputs["b_ih_b"], inputs["b_hh_b"])])
    whh_arr = np.stack([prep_whh(inputs["w_hh_f"]), prep_whh(inputs["w_hh_b"])])

    in_maps = []
    for c in range(NCORES):
        if c == 0:
            w = np.concatenate([seq[0:1], seq[0:TOK - 1]])
        else:
            w = seq[TOK * c - 1: TOK * c + TOK - 1]
        v = w.reshape(COLS, SEG_LEN).T.copy()   # v[l, n] = token for (step l, col n)
        if c == 0:
            v[:, 0] = seq[0:SEG_LEN]            # col 0: seq[0..7], step 7 masked
        # idx32[p, j] = token for gather j, partition p (k = j*128+p in
        # (l, n) order: l = j//4, n = (j%4)*128 + p)
        wrap = np.ascontiguousarray(
            v.reshape(TOK // 128, 128).T).astype(np.int32)

        m = np.zeros((2, 2 * COLS), dtype=np.float32)
        m[0, :] = 1.0
        if c == 0:
            m[0, 0] = m[0, COLS] = 0.0          # keep-mask kills col 0 (both hid tiles)
            m[1, 0] = m[1, COLS] = 1.0
        in_maps.append({
            "emb": embp,
            "idx": wrap,
            "win": win_arr,
            "whh": whh_arr,
            "msk": m,
        })
    return in_maps


def kernel(**inputs) -> np.ndarray:
    from concourse import bass_utils

    if "nc" not in _CACHE:
        _CACHE["nc"] = _build_program()
    nc = _CACHE["nc"]

    in_maps = _prep_host(inputs)
    res = bass_utils.run_bass_kernel_spmd(nc, in_maps, core_ids=list(range(NCORES)))
    return np.concatenate([r["out"] for r in res.results], axis=0)


if __name__ == "__main__":
    nc = _build_program()
    print("program built ok")



# revision 2
# speedup vs baseline: 6.6226x; 6.6226x over previous
"""Trainium2 Bass kernel for nn_ColumnEncoding (bidirectional masked LSTM
over 4096 split-delimited token segments).

Sharding: data-parallel over the 4096 columns -> 512 columns per core on 8
cores.  The embedding table and LSTM weights are NOT replicated over the
host link: the host packs [bf16 table (304-wide rows) | input-gate weights |
recurrent weights] into one payload and uploads a distinct 1/8 row-slice to
each core (~2.76 MB/core instead of ~27 MB/core).  On device, a single
HBM->HBM AllGather over the NeuronLink fabric rebuilds the full payload on
every core, after which each core runs the identical SPMD program on its
512-column shard; the host concatenates the 8 [512, 512] outputs.

Per-core device pipeline:
  1. AllGather payload slice -> full 22 MB payload (table + weights) in DRAM.
  2. dma_gather(transpose=True) pulls this core's 4096 token embedding rows
     from the [VOCAB, 304] table region straight into X^T layout
     ([emb-elem -> 3 K-tiles of 128 partitions, tokens]) in (step, column)-
     major token order.  Table column 300 is constant 1.0, which
     materializes the bias row for the fused-bias matmul.
  3. For each step t (8) and direction (fwd l=t / bwd l=7-t), gates^T
     [1024, 512cols] are accumulated in PSUM as
        W_in_aug^T @ x_l  (3 K-tiles, bias via the ones row)
      + W_hh^T     @ h_{t-1} (2 K-tiles, skipped at t=0)
     in two 4-bank PSUM units ([i,f] and [o,g] after host-side gate row
     permutation i,f,o,g).
  4. ScalarE applies sigmoid over [i|f] (one 2048-wide op) and sigmoid/tanh
     over [o]/[g]; VectorE does the fp32 cell update; h is written bf16 and
     fed back as the next matmul rhs.
  5. The ragged first column (segment length 7 instead of 8) is handled with
     per-core mask data (masked-step h/c fixups), keeping the program SPMD.
  6. Final hidden states are PE-transposed to [cols, features] and DMA'd out
     as bf16 (cast to f32 on host).
"""

import numpy as np
import ml_dtypes

VOCAB = 32000
EMBED = 300
HID = 256
N_COLS = 4096
SEG_LEN = 8
T = N_COLS * SEG_LEN
NCORES = 8
COLS = N_COLS // NCORES          # 512 columns per core
TOK = COLS * SEG_LEN             # 4096 gathered tokens per core
W = 304                          # payload row width (bf16 elems, 608B)
KT_IN = 3                        # K tiles for the input matmul (128+128+45)
K_LAST = 45                      # valid K rows in the last input K-tile (256:300 + ones row)
KT_HH = 2                        # K tiles for the recurrent matmul (256 = 2*128)
G4 = 4 * HID                     # 1024 gates per direction

# packed payload geometry (elements over a [PAYR, W] bf16 tensor)
OFF_WIN = VOCAB * W              # 9_728_000 (table is exactly VOCAB rows)
WIN_ELEMS = 2 * 128 * KT_IN * G4  # 786_432
OFF_WHH = OFF_WIN + WIN_ELEMS    # 10_514_432
WHH_ELEMS = 2 * 128 * KT_HH * G4  # 524_288
PAY_END = OFF_WHH + WHH_ELEMS    # 11_038_720
PAYR = 36312                     # ceil(PAY_END / W / 8) * 8; 4539 rows/core
RPC = PAYR // NCORES             # payload rows per core

BF16 = ml_dtypes.bfloat16

_CACHE = {}


def _build_program(sim_full=False):
    """sim_full=True builds a single-core variant that takes the FULL
    payload as input and skips the AllGather (for CoreSim numerics)."""
    import concourse.bass as bass
    import concourse.mybir as mybir
    import concourse.tile as tile
    from concourse import bacc
    from concourse.masks import make_identity

    f32 = mybir.dt.float32
    bf16 = mybir.dt.bfloat16

    nc = bacc.Bacc("TRN2", target_bir_lowering=False, debug=False,
                   num_devices=(None if sim_full else NCORES))

    pay = nc.dram_tensor("pay", [PAYR if sim_full else RPC, W], bf16,
                         kind="ExternalInput").ap()
    idx = nc.dram_tensor("idx", [128, TOK // 128], mybir.dt.int32,
                         kind="ExternalInput").ap()
    msk = nc.dram_tensor("msk", [2, 2 * COLS], f32, kind="ExternalInput").ap()
    out = nc.dram_tensor("out", [COLS, 2 * HID], bf16, kind="ExternalOutput").ap()

    with tile.TileContext(nc) as tc:
        _body(tc, bass, mybir, make_identity, pay, idx, msk, out, sim_full)
    nc.compile()
    return nc


def _body(tc, bass, mybir, make_identity, pay, idx, msk, out, sim_full):
    nc = tc.nc
    f32 = mybir.dt.float32
    bf16 = mybir.dt.bfloat16
    SIG = mybir.ActivationFunctionType.Sigmoid
    TANH = mybir.ActivationFunctionType.Tanh
    F = 2 * COLS                 # free width of the [hid-tile, col] packed state

    with (
        tc.tile_pool(name="pdram", bufs=1, space="DRAM") as pdram,
        tc.tile_pool(name="singles", bufs=1) as singles,
        tc.tile_pool(name="gates", bufs=2, space="PSUM") as gp,
        tc.tile_pool(name="work", bufs=2) as work,
        tc.tile_pool(name="acts", bufs=3) as acts,
    ):
        # ---- rebuild the full payload (table + weights) on every core ----
        if sim_full:
            full = pay
        else:
            bounce = pdram.tile([RPC, W], bf16, name="bounce")
            fullt = pdram.tile([PAYR, W], bf16, name="fullt")
            nc.gpsimd.dma_start(out=bounce, in_=pay)
            nc.gpsimd.collective_compute(
                "AllGather", mybir.AluOpType.bypass,
                replica_groups=[list(range(NCORES))],
                ins=[bounce.opt()], outs=[fullt.opt()],
            )
            full = fullt.opt()

        # ---- constants / inputs to SBUF ----
        idx_sb = singles.tile([128, TOK // 128], mybir.dt.int32, name="idx_sb")
        nc.sync.dma_start(out=idx_sb, in_=idx)

        win_sb = []
        whh_sb = []
        for d in range(2):
            w1 = singles.tile([128, KT_IN * G4], bf16, name=f"win_sb{d}")
            src = bass.AP(tensor=full.tensor,
                          offset=full.offset + OFF_WIN + d * (128 * KT_IN * G4),
                          ap=[[KT_IN * G4, 128], [1, KT_IN * G4]])
            nc.sync.dma_start(out=w1, in_=src)
            win_sb.append(w1)
            w2 = singles.tile([128, KT_HH * G4], bf16, name=f"whh_sb{d}")
            src = bass.AP(tensor=full.tensor,
                          offset=full.offset + OFF_WHH + d * (128 * KT_HH * G4),
                          ap=[[KT_HH * G4, 128], [1, KT_HH * G4]])
            nc.sync.dma_start(out=w2, in_=src)
            whh_sb.append(w2)

        # broadcast per-core masks to all 128 partitions
        def bcast_row(r, name):
            t = singles.tile([128, F], f32, name=name)
            src = bass.AP(tensor=msk.tensor, offset=msk.offset + r * F,
                          ap=[[0, 128], [1, F]])
            nc.gpsimd.dma_start(out=t, in_=src)
            return t

        K32 = bcast_row(0, "K32")     # keep mask: 0 at core-0 col 0, else 1
        M32 = bcast_row(1, "M32")     # 1 - keep
        Kbf = singles.tile([128, F], bf16, name="Kbf")
        nc.vector.tensor_copy(Kbf, K32)

        ident = singles.tile([128, 128], f32, name="ident")
        make_identity(nc, ident)

        # table view: payload rows [0, VOCAB) hold the [VOCAB, W] bf16 table
        emb = bass.AP(tensor=full.tensor, offset=full.offset,
                      ap=[[W, VOCAB], [1, W]])

        # ---- gather X^T per step: XT[l][p, kt, n] = emb_row(tok[l,n])[kt*128+p]
        # indirect row gathers (128 rows/op) -> per-l DRAM staging -> DMA
        # transposes back into [emb-elem, token] K-tile layout.
        XT = [None] * SEG_LEN
        with tc.tile_pool(name="gx", bufs=4) as gxp, \
             tc.tile_pool(name="xd", bufs=1, space="DRAM") as xdp:
            for l in (0, 7, 1, 6, 2, 5, 3, 4):
                xd = xdp.tile([COLS, W], bf16, name=f"xd_{l}", tag=f"xd{l}")
                for jj in range(COLS // 128):
                    j = l * (COLS // 128) + jj
                    xg = gxp.tile([128, W], bf16, name=f"xg{l}_{jj}", tag="xg")
                    nc.gpsimd.indirect_dma_start(
                        out=xg,
                        out_offset=None,
                        in_=emb,
                        in_offset=bass.IndirectOffsetOnAxis(
                            ap=idx_sb[:, j:j + 1], axis=0),
                    )
                    nc.sync.dma_start(out=xd[jj * 128:(jj + 1) * 128, :],
                                      in_=xg)
                xt = singles.tile([128, KT_IN, COLS], bf16,
                                  name=f"xt_{l}", tag=f"xt{l}")
                for kt in range(KT_IN):
                    k0 = kt * 128
                    k1 = min(k0 + 128, W)
                    nc.sync.dma_start_transpose(
                        out=xt[0:k1 - k0, kt, :], in_=xd[:, k0:k1])
                XT[l] = xt

        # ---- recurrence ----
        h_prev = [None, None]        # bf16 [128, F] per direction
        c_prev = [None, None]        # f32  [128, F] per direction
        h_fin32 = [None, None]       # final fp32 hidden per direction
        h6_32 = None                 # fwd h after step 6 (col-0 ragged fix)

        for t in range(SEG_LEN):
            for d in range(2):       # 0 = fwd, 1 = bwd
                l = t if d == 0 else SEG_LEN - 1 - t
                units = []
                for ui in range(2):  # unit 0: gates [i|f], unit 1: [o|g]
                    u = gp.tile([128, 4 * COLS], f32, name=f"u{t}_{d}_{ui}",
                                tag="u")
                    for mi in range(4):
                        m = ui * 4 + mi
                        dst = u[:, mi * COLS:(mi + 1) * COLS]
                        for kt in range(KT_IN):
                            kp = K_LAST if kt == KT_IN - 1 else 128
                            nc.tensor.matmul(
                                dst,
                                win_sb[d][0:kp, kt * G4 + m * 128:kt * G4 + (m + 1) * 128],
                                XT[l][0:kp, kt, :],
                                start=(kt == 0),
                                stop=(kt == KT_IN - 1 and t == 0),
                            )
                        if t > 0:
                            for kt in range(KT_HH):
                                nc.tensor.matmul(
                                    dst,
                                    whh_sb[d][:, kt * G4 + m * 128:kt * G4 + (m + 1) * 128],
                                    h_prev[d][:, kt * COLS:(kt + 1) * COLS],
                                    start=False,
                                    stop=(kt == KT_HH - 1),
                                )
                    units.append(u)

                s1 = acts.tile([128, 4 * COLS], f32, name=f"s1_{t}_{d}", tag="s1")
                nc.scalar.activation(s1, units[0][:, :], SIG)
                so = acts.tile([128, F], f32, name=f"so_{t}_{d}", tag="so")
                nc.scalar.activation(so, units[1][:, 0:F], SIG)
                tg = acts.tile([128, F], f32, name=f"tg_{t}_{d}", tag="tg")
                nc.scalar.activation(tg, units[1][:, F:2 * F], TANH)

                # cell update (fp32): c = sig_f * c + sig_i * tanh_g
                t2 = work.tile([128, F], f32, name=f"t2_{t}_{d}", tag="t2")
                nc.vector.tensor_mul(t2, s1[:, 0:F], tg)
                if t == 0:
                    c_new = t2
                else:
                    t1 = work.tile([128, F], f32, name=f"t1_{t}_{d}", tag="t1")
                    nc.vector.tensor_mul(t1, s1[:, F:2 * F], c_prev[d])
                    c_new = work.tile([128, F], f32, name=f"c_{t}_{d}", tag=f"c{d}")
                    nc.vector.tensor_add(c_new, t1, t2)

                tc_ = acts.tile([128, F], f32, name=f"tc_{t}_{d}", tag="tc")
                nc.scalar.activation(tc_, c_new, TANH)

                h_bf = work.tile([128, F], bf16, name=f"h_{t}_{d}", tag=f"h{d}")
                nc.vector.tensor_mul(h_bf, so, tc_)

                if d == 1 and t == 0:
                    # bwd step 0 is masked for (core 0) column 0: zero h, c
                    cm = work.tile([128, F], f32, name="c_bm", tag=f"c{d}")
                    nc.vector.tensor_mul(cm, c_new, K32)
                    c_new = cm
                    hm = work.tile([128, F], bf16, name="h_bm", tag=f"h{d}")
                    nc.vector.tensor_mul(hm, h_bf, Kbf)
                    h_bf = hm

                if d == 0 and t == SEG_LEN - 2:
                    # fwd h after step 6, fp32 (output for the ragged column 0)
                    h6_32 = work.tile([128, F], f32, name="h6_32", tag="hf32",
                                      bufs=6)
                    nc.vector.tensor_mul(h6_32, so, tc_)
                if t == SEG_LEN - 1:
                    hf = work.tile([128, F], f32, name=f"hfin{d}", tag="hf32",
                                   bufs=6)
                    nc.vector.tensor_mul(hf, so, tc_)
                    h_fin32[d] = hf

                c_prev[d] = c_new
                h_prev[d] = h_bf

        # fwd ragged fix: column 0 of core 0 takes the step-6 hidden state
        # (blend: h7*K + h6*(1-K); avoids copy_predicated's int-mask needs)
        b1 = work.tile([128, F], f32, name="b1", tag="hf32", bufs=6)
        nc.vector.tensor_mul(b1, h_fin32[0], K32)
        b2 = work.tile([128, F], f32, name="b2", tag="hf32", bufs=6)
        nc.vector.tensor_mul(b2, h6_32, M32)
        hf_sel = work.tile([128, F], f32, name="hf_sel", tag="hf32", bufs=6)
        nc.vector.tensor_add(hf_sel, b1, b2)
        h_fin32[0] = hf_sel

        # ---- transpose [hid, col] -> [col, feat] and write out (bf16) ----
        out_t = []
        for nt in range(COLS // 128):
            o = singles.tile([128, 2 * HID], bf16, name=f"out_t{nt}")
            out_t.append(o)
        for d in range(2):
            for ht in range(2):
                for nt in range(COLS // 128):
                    tp = gp.tile([128, 128], f32, name=f"tp{d}_{ht}_{nt}", tag="u")
                    nc.tensor.transpose(
                        tp, h_fin32[d][:, ht * COLS + nt * 128:ht * COLS + (nt + 1) * 128],
                        ident)
                    nc.vector.tensor_copy(
                        out_t[nt][:, d * HID + ht * 128:d * HID + (ht + 1) * 128], tp)
        for nt in range(COLS // 128):
            nc.sync.dma_start(out=out[nt * 128:(nt + 1) * 128, :], in_=out_t[nt])


def _prep_host(inputs, sim_full=False):
    """Build the per-core input maps from the full problem inputs."""
    emb_table = np.asarray(inputs["emb_table"], dtype=np.float32)
    seq = np.asarray(inputs["seq_s"]).astype(np.int64)

    perm = np.concatenate([np.arange(0, 2 * HID),            # i, f
                           np.arange(3 * HID, 4 * HID),      # o
                           np.arange(2 * HID, 3 * HID)])     # g

    def prep_win(w_ih, b_ih, b_hh):
        aug = np.zeros((G4, KT_IN * 128), dtype=np.float32)
        aug[:, :EMBED] = np.asarray(w_ih, np.float32)
        aug[:, EMBED] = np.asarray(b_ih, np.float32) + np.asarray(b_hh, np.float32)
        aug = aug[perm]
        a = aug.T.reshape(KT_IN, 128, G4).transpose(1, 0, 2)
        return np.ascontiguousarray(a.reshape(128, KT_IN * G4)).astype(BF16)

    def prep_whh(w_hh):
        a = np.asarray(w_hh, np.float32)[perm].T.reshape(KT_HH, 128, G4)
        return np.ascontiguousarray(
            a.transpose(1, 0, 2).reshape(128, KT_HH * G4)).astype(BF16)

    # packed payload: [bf16 table (W-wide rows, ones col at 300) | win | whh]
    pay = np.zeros(PAYR * W, dtype=BF16)
    tab = pay[:VOCAB * W].reshape(VOCAB, W)
    tab[:, :EMBED] = emb_table.astype(BF16)
    tab[:, EMBED] = 1.0  # ones column -> bias row of X^T
    pay[OFF_WIN:OFF_WIN + WIN_ELEMS // 2] = prep_win(
        inputs["w_ih_f"], inputs["b_ih_f"], inputs["b_hh_f"]).ravel()
    pay[OFF_WIN + WIN_ELEMS // 2:OFF_WHH] = prep_win(
        inputs["w_ih_b"], inputs["b_ih_b"], inputs["b_hh_b"]).ravel()
    pay[OFF_WHH:OFF_WHH + WHH_ELEMS // 2] = prep_whh(inputs["w_hh_f"]).ravel()
    pay[OFF_WHH + WHH_ELEMS // 2:PAY_END] = prep_whh(inputs["w_hh_b"]).ravel()
    pay2d = pay.reshape(PAYR, W)

    in_maps = []
    for c in range(NCORES):
        if c == 0:
            w = np.concatenate([seq[0:1], seq[0:TOK - 1]])
        else:
            w = seq[TOK * c - 1: TOK * c + TOK - 1]
        v = w.reshape(COLS, SEG_LEN).T.copy()   # v[l, n] = token for (step l, col n)
        if c == 0:
            v[:, 0] = seq[0:SEG_LEN]            # col 0: seq[0..7], step 7 masked
        # idx32[p, j] = token for gather j, partition p (k = j*128+p in
        # (l, n) order: l = j//4, n = (j%4)*128 + p)
        wrap = np.ascontiguousarray(
            v.reshape(TOK // 128, 128).T).astype(np.int32)

        m = np.zeros((2, 2 * COLS), dtype=np.float32)
        m[0, :] = 1.0
        if c == 0:
            m[0, 0] = m[0, COLS] = 0.0          # keep-mask kills col 0 (both hid tiles)
            m[1, 0] = m[1, COLS] = 1.0
        in_maps.append({
            "pay": pay2d if sim_full else pay2d[c * RPC:(c + 1) * RPC],
            "idx": wrap,
            "msk": m,
        })
    return in_maps


def kernel(**inputs) -> np.ndarray:
    from concourse import bass_utils

    if "nc" not in _CACHE:
        _CACHE["nc"] = _build_program()
    nc = _CACHE["nc"]

    in_maps = _prep_host(inputs)
    res = bass_utils.run_bass_kernel_spmd(nc, in_maps, core_ids=list(range(NCORES)))
    return np.concatenate(
        [r["out"].astype(np.float32) for r in res.results], axis=0)


if __name__ == "__main__":
    nc = _build_program()
    print("program built ok")


# revision 7
# speedup vs baseline: 7.9985x; 1.2078x over previous
"""Trainium2 Bass kernel for nn_ColumnEncoding (bidirectional masked LSTM
over 4096 split-delimited token segments).

Sharding: data-parallel over the 4096 columns -> 512 columns per core on 8
cores.  The embedding table and LSTM weights are NOT replicated over the
host link: the host packs [bf16 table (304-wide rows) | input-gate weights |
recurrent weights] into one payload and uploads a distinct 1/8 row-slice to
each core (~2.76 MB/core instead of ~27 MB/core).  On device, a single
HBM->HBM AllGather over the NeuronLink fabric rebuilds the full payload on
every core, after which each core runs the identical SPMD program on its
512-column shard; the host concatenates the 8 [512, 512] outputs.

Per-core device pipeline:
  1. AllGather payload slice -> full 22 MB payload (table + weights) in DRAM.
  2. dma_gather(transpose=True) pulls this core's 4096 token embedding rows
     from the [VOCAB, 304] table region straight into X^T layout
     ([emb-elem -> 3 K-tiles of 128 partitions, tokens]) in (step, column)-
     major token order.  Table column 300 is constant 1.0, which
     materializes the bias row for the fused-bias matmul.
  3. For each step t (8) and direction (fwd l=t / bwd l=7-t), gates^T
     [1024, 512cols] are accumulated in PSUM as
        W_in_aug^T @ x_l  (3 K-tiles, bias via the ones row)
      + W_hh^T     @ h_{t-1} (2 K-tiles, skipped at t=0)
     in two 4-bank PSUM units ([i,f] and [o,g] after host-side gate row
     permutation i,f,o,g).
  4. ScalarE applies sigmoid over [i|f] (one 2048-wide op) and sigmoid/tanh
     over [o]/[g]; VectorE does the fp32 cell update; h is written bf16 and
     fed back as the next matmul rhs.
  5. The ragged first column (segment length 7 instead of 8) is handled with
     per-core mask data (masked-step h/c fixups), keeping the program SPMD.
  6. Final hidden states are PE-transposed to [cols, features] and DMA'd out
     as bf16 (cast to f32 on host).
"""

import numpy as np
import ml_dtypes

VOCAB = 32000
EMBED = 300
HID = 256
N_COLS = 4096
SEG_LEN = 8
T = N_COLS * SEG_LEN
NCORES = 8
COLS = N_COLS // NCORES          # 512 columns per core
TOK = COLS * SEG_LEN             # 4096 gathered tokens per core
W = 304                          # payload row width (bf16 elems, 608B)
KT_IN = 3                        # K tiles for the input matmul (128+128+45)
K_LAST = 45                      # valid K rows in the last input K-tile (256:300 + ones row)
KT_HH = 2                        # K tiles for the recurrent matmul (256 = 2*128)
G4 = 4 * HID                     # 1024 gates per direction

# packed payload geometry (elements over a [payr, W] bf16 tensor).  The
# table region holds only the embedding rows this input actually uses
# (~20.6k unique tokens of the 32k vocab), compacted host-side; R_TAB is a
# >20-sigma upper bound on the unique count for uniform tokens.  If an input
# ever exceeds it, kernel() falls back to a full-vocab program variant.
WIN_ELEMS = 2 * 128 * KT_IN * G4  # 786_432
WHH_ELEMS = 2 * 128 * KT_HH * G4  # 524_288
R_TAB = 22528


def _geom(r_tab):
    off_win = r_tab * W
    off_whh = off_win + WIN_ELEMS
    pay_end = off_whh + WHH_ELEMS
    payr = -(-pay_end // W)
    payr = -(-payr // NCORES) * NCORES
    return off_win, off_whh, pay_end, payr


BF16 = ml_dtypes.bfloat16

_CACHE = {}


def _build_program(sim_full=False, r_tab=R_TAB):
    """sim_full=True builds a single-core variant that takes the FULL
    payload as input and skips the AllGather (for CoreSim numerics)."""
    import concourse.bass as bass
    import concourse.mybir as mybir
    import concourse.tile as tile
    from concourse import bacc
    from concourse.masks import make_identity

    f32 = mybir.dt.float32
    bf16 = mybir.dt.bfloat16

    nc = bacc.Bacc("TRN2", target_bir_lowering=False, debug=False,
                   num_devices=(None if sim_full else NCORES))

    _, _, _, payr = _geom(r_tab)
    pay = nc.dram_tensor("pay", [payr if sim_full else payr // NCORES, W],
                         bf16, kind="ExternalInput").ap()
    idx = nc.dram_tensor("idx", [128, TOK // 128], mybir.dt.int32,
                         kind="ExternalInput").ap()
    msk = nc.dram_tensor("msk", [2, 2 * COLS], f32, kind="ExternalInput").ap()
    out = nc.dram_tensor("out", [COLS, 2 * HID], bf16, kind="ExternalOutput").ap()

    with tile.TileContext(nc) as tc:
        _body(tc, bass, mybir, make_identity, pay, idx, msk, out, sim_full,
              r_tab)
    nc.compile()
    return nc


def _body(tc, bass, mybir, make_identity, pay, idx, msk, out, sim_full,
          r_tab):
    nc = tc.nc
    OFF_WIN, OFF_WHH, _, PAYR = _geom(r_tab)
    RPC = PAYR // NCORES
    f32 = mybir.dt.float32
    bf16 = mybir.dt.bfloat16
    SIG = mybir.ActivationFunctionType.Sigmoid
    TANH = mybir.ActivationFunctionType.Tanh
    F = 2 * COLS                 # free width of the [hid-tile, col] packed state

    with (
        tc.tile_pool(name="pdram", bufs=1, space="DRAM") as pdram,
        tc.tile_pool(name="singles", bufs=1) as singles,
        tc.tile_pool(name="gates", bufs=2, space="PSUM") as gp,
        tc.tile_pool(name="work", bufs=2) as work,
        tc.tile_pool(name="acts", bufs=3) as acts,
    ):
        # ---- rebuild the full payload (table + weights) on every core ----
        if sim_full:
            full = pay
        else:
            bounce = pdram.tile([RPC, W], bf16, name="bounce")
            fullt = pdram.tile([PAYR, W], bf16, name="fullt")
            nc.gpsimd.dma_start(out=bounce, in_=pay)
            nc.gpsimd.collective_compute(
                "AllGather", mybir.AluOpType.bypass,
                replica_groups=[list(range(NCORES))],
                ins=[bounce.opt()], outs=[fullt.opt()],
            )
            full = fullt.opt()

        # ---- constants / inputs to SBUF ----
        idx_sb = singles.tile([128, TOK // 128], mybir.dt.int32, name="idx_sb")
        nc.sync.dma_start(out=idx_sb, in_=idx)

        win_sb = []
        whh_sb = []
        for d in range(2):
            w1 = singles.tile([128, KT_IN * G4], bf16, name=f"win_sb{d}")
            src = bass.AP(tensor=full.tensor,
                          offset=full.offset + OFF_WIN + d * (128 * KT_IN * G4),
                          ap=[[KT_IN * G4, 128], [1, KT_IN * G4]])
            nc.sync.dma_start(out=w1, in_=src)
            win_sb.append(w1)
            w2 = singles.tile([128, KT_HH * G4], bf16, name=f"whh_sb{d}")
            src = bass.AP(tensor=full.tensor,
                          offset=full.offset + OFF_WHH + d * (128 * KT_HH * G4),
                          ap=[[KT_HH * G4, 128], [1, KT_HH * G4]])
            nc.sync.dma_start(out=w2, in_=src)
            whh_sb.append(w2)

        # broadcast per-core masks to all 128 partitions
        def bcast_row(r, name):
            t = singles.tile([128, F], f32, name=name)
            src = bass.AP(tensor=msk.tensor, offset=msk.offset + r * F,
                          ap=[[0, 128], [1, F]])
            nc.gpsimd.dma_start(out=t, in_=src)
            return t

        K32 = bcast_row(0, "K32")     # keep mask: 0 at core-0 col 0, else 1
        M32 = bcast_row(1, "M32")     # 1 - keep
        Kbf = singles.tile([128, F], bf16, name="Kbf")
        nc.vector.tensor_copy(Kbf, K32)

        ident = singles.tile([128, 128], f32, name="ident")
        make_identity(nc, ident)

        # table view: payload rows [0, r_tab) hold the compacted bf16 table
        emb = bass.AP(tensor=full.tensor, offset=full.offset,
                      ap=[[W, r_tab], [1, W]])

        # ---- gather X^T per step: XT[l][p, kt, n] = emb_row(tok[l,n])[kt*128+p]
        # indirect row gathers (128 rows/op) -> per-l DRAM staging -> DMA
        # transposes back into [emb-elem, token] K-tile layout.
        XT = [None] * SEG_LEN
        with tc.tile_pool(name="gx", bufs=4) as gxp, \
             tc.tile_pool(name="xd", bufs=1, space="DRAM") as xdp:
            for l in (0, 7, 1, 6, 2, 5, 3, 4):
                xd = xdp.tile([COLS, W], bf16, name=f"xd_{l}", tag=f"xd{l}")
                for jj in range(COLS // 128):
                    j = l * (COLS // 128) + jj
                    xg = gxp.tile([128, W], bf16, name=f"xg{l}_{jj}", tag="xg")
                    nc.gpsimd.indirect_dma_start(
                        out=xg,
                        out_offset=None,
                        in_=emb,
                        in_offset=bass.IndirectOffsetOnAxis(
                            ap=idx_sb[:, j:j + 1], axis=0),
                    )
                    nc.sync.dma_start(out=xd[jj * 128:(jj + 1) * 128, :],
                                      in_=xg)
                xt = singles.tile([128, KT_IN, COLS], bf16,
                                  name=f"xt_{l}", tag=f"xt{l}")
                for kt in range(KT_IN):
                    k0 = kt * 128
                    k1 = min(k0 + 128, W)
                    nc.sync.dma_start_transpose(
                        out=xt[0:k1 - k0, kt, :], in_=xd[:, k0:k1])
                XT[l] = xt

        # ---- recurrence ----
        h_prev = [None, None]        # bf16 [128, F] per direction
        c_prev = [None, None]        # f32  [128, F] per direction
        h_fin32 = [None, None]       # final fp32 hidden per direction
        h6_32 = None                 # fwd h after step 6 (col-0 ragged fix)

        for t in range(SEG_LEN):
            for d in range(2):       # 0 = fwd, 1 = bwd
                l = t if d == 0 else SEG_LEN - 1 - t
                units = []
                for ui in range(2):  # unit 0: gates [i|f], unit 1: [o|g]
                    u = gp.tile([128, 4 * COLS], f32, name=f"u{t}_{d}_{ui}",
                                tag="u")
                    for mi in range(4):
                        m = ui * 4 + mi
                        dst = u[:, mi * COLS:(mi + 1) * COLS]
                        for kt in range(KT_IN):
                            kp = K_LAST if kt == KT_IN - 1 else 128
                            nc.tensor.matmul(
                                dst,
                                win_sb[d][0:kp, kt * G4 + m * 128:kt * G4 + (m + 1) * 128],
                                XT[l][0:kp, kt, :],
                                start=(kt == 0),
                                stop=(kt == KT_IN - 1 and t == 0),
                            )
                        if t > 0:
                            for kt in range(KT_HH):
                                nc.tensor.matmul(
                                    dst,
                                    whh_sb[d][:, kt * G4 + m * 128:kt * G4 + (m + 1) * 128],
                                    h_prev[d][:, kt * COLS:(kt + 1) * COLS],
                                    start=False,
                                    stop=(kt == KT_HH - 1),
                                )
                    units.append(u)

                s1 = acts.tile([128, 4 * COLS], f32, name=f"s1_{t}_{d}", tag="s1")
                nc.scalar.activation(s1, units[0][:, :], SIG)
                so = acts.tile([128, F], f32, name=f"so_{t}_{d}", tag="so")
                nc.scalar.activation(so, units[1][:, 0:F], SIG)
                tg = acts.tile([128, F], f32, name=f"tg_{t}_{d}", tag="tg")
                nc.scalar.activation(tg, units[1][:, F:2 * F], TANH)

                # cell update (fp32): c = sig_f * c + sig_i * tanh_g
                t2 = work.tile([128, F], f32, name=f"t2_{t}_{d}", tag="t2")
                nc.vector.tensor_mul(t2, s1[:, 0:F], tg)
                if t == 0:
                    c_new = t2
                else:
                    t1 = work.tile([128, F], f32, name=f"t1_{t}_{d}", tag="t1")
                    nc.vector.tensor_mul(t1, s1[:, F:2 * F], c_prev[d])
                    c_new = work.tile([128, F], f32, name=f"c_{t}_{d}", tag=f"c{d}")
                    nc.vector.tensor_add(c_new, t1, t2)

                tc_ = acts.tile([128, F], f32, name=f"tc_{t}_{d}", tag="tc")
                nc.scalar.activation(tc_, c_new, TANH)

                h_bf = work.tile([128, F], bf16, name=f"h_{t}_{d}", tag=f"h{d}")
                nc.vector.tensor_mul(h_bf, so, tc_)

                if d == 1 and t == 0:
                    # bwd step 0 is masked for (core 0) column 0: zero h, c
                    cm = work.tile([128, F], f32, name="c_bm", tag=f"c{d}")
                    nc.vector.tensor_mul(cm, c_new, K32)
                    c_new = cm
                    hm = work.tile([128, F], bf16, name="h_bm", tag=f"h{d}")
                    nc.vector.tensor_mul(hm, h_bf, Kbf)
                    h_bf = hm

                if d == 0 and t == SEG_LEN - 2:
                    # fwd h after step 6, fp32 (output for the ragged column 0)
                    h6_32 = work.tile([128, F], f32, name="h6_32", tag="hf32",
                                      bufs=6)
                    nc.vector.tensor_mul(h6_32, so, tc_)
                if t == SEG_LEN - 1:
                    hf = work.tile([128, F], f32, name=f"hfin{d}", tag="hf32",
                                   bufs=6)
                    nc.vector.tensor_mul(hf, so, tc_)
                    h_fin32[d] = hf

                c_prev[d] = c_new
                h_prev[d] = h_bf

        # fwd ragged fix: column 0 of core 0 takes the step-6 hidden state
        # (blend: h7*K + h6*(1-K); avoids copy_predicated's int-mask needs)
        b1 = work.tile([128, F], f32, name="b1", tag="hf32", bufs=6)
        nc.vector.tensor_mul(b1, h_fin32[0], K32)
        b2 = work.tile([128, F], f32, name="b2", tag="hf32", bufs=6)
        nc.vector.tensor_mul(b2, h6_32, M32)
        hf_sel = work.tile([128, F], f32, name="hf_sel", tag="hf32", bufs=6)
        nc.vector.tensor_add(hf_sel, b1, b2)
        h_fin32[0] = hf_sel

        # ---- transpose [hid, col] -> [col, feat] and write out (bf16) ----
        out_t = []
        for nt in range(COLS // 128):
            o = singles.tile([128, 2 * HID], bf16, name=f"out_t{nt}")
            out_t.append(o)
        for d in range(2):
            for ht in range(2):
                for nt in range(COLS // 128):
                    tp = gp.tile([128, 128], f32, name=f"tp{d}_{ht}_{nt}", tag="u")
                    nc.tensor.transpose(
                        tp, h_fin32[d][:, ht * COLS + nt * 128:ht * COLS + (nt + 1) * 128],
                        ident)
                    nc.vector.tensor_copy(
                        out_t[nt][:, d * HID + ht * 128:d * HID + (ht + 1) * 128], tp)
        for nt in range(COLS // 128):
            nc.sync.dma_start(out=out[nt * 128:(nt + 1) * 128, :], in_=out_t[nt])


def _prep_host(inputs, sim_full=False, r_tab=R_TAB):
    """Build the per-core input maps from the full problem inputs.

    Returns None if the input needs more than r_tab distinct embedding rows
    (caller then retries with r_tab=VOCAB, where tokens index the table
    directly and no compaction happens).
    """
    emb_table = np.asarray(inputs["emb_table"], dtype=np.float32)
    seq = np.asarray(inputs["seq_s"]).astype(np.int64)

    if r_tab >= VOCAB:
        seqc = seq
        n_rows = VOCAB
        rows = slice(None)
    else:
        uniq, inv = np.unique(seq, return_inverse=True)
        if len(uniq) > r_tab:
            return None
        seqc = inv.astype(np.int64)   # remapped tokens index compacted rows
        n_rows = len(uniq)
        rows = uniq

    perm = np.concatenate([np.arange(0, 2 * HID),            # i, f
                           np.arange(3 * HID, 4 * HID),      # o
                           np.arange(2 * HID, 3 * HID)])     # g

    def prep_win(w_ih, b_ih, b_hh):
        aug = np.zeros((G4, KT_IN * 128), dtype=np.float32)
        aug[:, :EMBED] = np.asarray(w_ih, np.float32)
        aug[:, EMBED] = np.asarray(b_ih, np.float32) + np.asarray(b_hh, np.float32)
        aug = aug[perm]
        a = aug.T.reshape(KT_IN, 128, G4).transpose(1, 0, 2)
        return np.ascontiguousarray(a.reshape(128, KT_IN * G4)).astype(BF16)

    def prep_whh(w_hh):
        a = np.asarray(w_hh, np.float32)[perm].T.reshape(KT_HH, 128, G4)
        return np.ascontiguousarray(
            a.transpose(1, 0, 2).reshape(128, KT_HH * G4)).astype(BF16)

    # packed payload: [bf16 table (W-wide rows, ones col at 300) | win | whh]
    OFF_WIN, OFF_WHH, PAY_END, PAYR = _geom(r_tab)
    RPC = PAYR // NCORES
    pay = np.zeros(PAYR * W, dtype=BF16)
    tab = pay[:r_tab * W].reshape(r_tab, W)
    tab[:n_rows, :EMBED] = emb_table[rows].astype(BF16)
    tab[:n_rows, EMBED] = 1.0  # ones column -> bias row of X^T
    pay[OFF_WIN:OFF_WIN + WIN_ELEMS // 2] = prep_win(
        inputs["w_ih_f"], inputs["b_ih_f"], inputs["b_hh_f"]).ravel()
    pay[OFF_WIN + WIN_ELEMS // 2:OFF_WHH] = prep_win(
        inputs["w_ih_b"], inputs["b_ih_b"], inputs["b_hh_b"]).ravel()
    pay[OFF_WHH:OFF_WHH + WHH_ELEMS // 2] = prep_whh(inputs["w_hh_f"]).ravel()
    pay[OFF_WHH + WHH_ELEMS // 2:PAY_END] = prep_whh(inputs["w_hh_b"]).ravel()
    pay2d = pay.reshape(PAYR, W)

    in_maps = []
    for c in range(NCORES):
        if c == 0:
            w = np.concatenate([seqc[0:1], seqc[0:TOK - 1]])
        else:
            w = seqc[TOK * c - 1: TOK * c + TOK - 1]
        v = w.reshape(COLS, SEG_LEN).T.copy()   # v[l, n] = token for (step l, col n)
        if c == 0:
            v[:, 0] = seqc[0:SEG_LEN]           # col 0: seq[0..7], step 7 masked
        # idx32[p, j] = token for gather j, partition p (k = j*128+p in
        # (l, n) order: l = j//4, n = (j%4)*128 + p)
        wrap = np.ascontiguousarray(
            v.reshape(TOK // 128, 128).T).astype(np.int32)

        m = np.zeros((2, 2 * COLS), dtype=np.float32)
        m[0, :] = 1.0
        if c == 0:
            m[0, 0] = m[0, COLS] = 0.0          # keep-mask kills col 0 (both hid tiles)
            m[1, 0] = m[1, COLS] = 1.0
        in_maps.append({
            "pay": pay2d if sim_full else pay2d[c * RPC:(c + 1) * RPC],
            "idx": wrap,
            "msk": m,
        })
    return in_maps


def kernel(**inputs) -> np.ndarray:
    from concourse import bass_utils

    for r_tab in (R_TAB, VOCAB):
        in_maps = _prep_host(inputs, r_tab=r_tab)
        if in_maps is not None:
            break
    key = f"nc{r_tab}"
    if key not in _CACHE:
        _CACHE[key] = _build_program(r_tab=r_tab)
    nc = _CACHE[key]

    res = bass_utils.run_bass_kernel_spmd(nc, in_maps, core_ids=list(range(NCORES)))
    return np.concatenate(
        [r["out"].astype(np.float32) for r in res.results], axis=0)


if __name__ == "__main__":
    nc = _build_program()
    print("program built ok")
